# revision 20
# baseline (speedup 1.0000x reference)
"""Trainium2 Bass kernel for nn_Diffusion_15436112462451.

Strategy: pure data parallelism over the batch (2048 -> 8 cores x 256),
feature-major activations on-chip, and -- the key change vs the unrolled
baseline -- the 100-step denoising loop runs as a single For_i HARDWARE
loop.  The execution environment charges a large fixed cost per STATIC
program instruction (~60us each; measured: an unrolled 2900-instruction
program costs ~183ms while the same work inside a hardware loop is
dominated by true device time ~1ms).  The loop body is ~29 static
instructions; all step-varying quantities are indexed with register-based
dynamic APs (bass.ds) off per-step SBUF tables:

  - noise        [16, 100, BPC] f32, slice [:, ds(k,1), :]
  - temb contrib [128, 100] f32 x2 chunks, column ds(k,1) added to the L1
    PSUM with a per-partition tensor_scalar add (f32 -- exact bias)
  - schedule scalars c1/-c2/p1/p2: [16, 100] f32 tables, column ds(k,1),
    consumed as per-partition scalar APs by the x-update DVE ops

Per step: 12 bf16 matmuls on PE (L1 K=80 x2, L2/L3 4 each, L4 2; biases
b2/b3/b4 ride a mask rank-4 matmul / rank-2 prime only when nonzero --
they are zero in this problem), 2 temb-column adds + 3 sigmoid passes on
ScalarE/DVE, 6 custom-DVE mish-completion passes (exact-mish quintic in
t = sigmoid(-az-d)^2, max err 6.3e-5), and a 4-op x-update on DVE
(s2 / PREOP2 / CLIPMA2 / bf16-copy; the Pool engine rejects the
register-pointer TensorScalarPtr variants, and an in-place ScalarE
Identity-with-dynamic-bias on PSUM computes wrong results -- both
learned the hard way, keep these on the DVE).  The x iterate is kept in
f32 (bf16-only x fails: rel err 5.5e-2 vs 8.5e-4 with f32 master).  The
time-embedding MLP is batch-independent and precomputed on the host into
the [100,256] contrib table.  All constant inputs ride in two packed
blobs (bf16 weights+state / f32 tables) = 4 preamble DMAs total;
static program size is ~199 instructions (vs ~2950 unrolled), which
dominates the measured device span in this environment.
"""
import sys
import math
import re
import numpy as np

for _p in ('/opt/trn_rl_repo', '/root/.axon_site/_ro/trn_rl_repo'):
    if _p not in sys.path:
        sys.path.insert(0, _p)

import ml_dtypes
from contextlib import ExitStack
import concourse.bass as bass
from concourse import bacc
from concourse import mybir, tile, bass_utils, dve_ops
from concourse.dve_spec import Spec, Src0, Src1, C0, C1, C2, sq, maxx, minn

BF16 = ml_dtypes.bfloat16
NCORES = 8
BATCH = 2048
BPC = BATCH // NCORES          # 256 batch rows per core
T_STEPS = 100
STATE_DIM, ACTION_DIM, HIDDEN, TIME_DIM = 64, 16, 256, 32
KX = ACTION_DIM + STATE_DIM    # 80 rows of W1 used for [x; state]

# --- activation fit constants (deg-5 sigma-poly factorization) ---
A_S = 0.9990298806699722
D_S = -0.0005000143935776705
BETA = 4.708088756431602e-05
QA, QB, QC = -0.21302398380145082, 0.6455208072356895, -0.6201860532189531
MA, MB, MC = -0.9194163848641597, 1.5334239721923986, -1.6124382654378613


# ---------------------------------------------------------------- custom ops
def _register_op(name, spec):
    for op in dve_ops.OPS:
        if op.name == name:
            return op
    op = dve_ops.DveOp(name, spec, False, uops_sha={"v3": "?", "v4": "?"})
    dve_ops.OPS.append(op)
    dve_ops.CUSTOM_DVE_SPECS[name] = spec
    dve_ops._SUB_OPCODE_FOR_NAME[name] = (
        dve_ops._CUSTOM_DVE_ROW_BASE + len(dve_ops.OPS) - 1)
    for ver in ("v3", "v4"):
        try:
            op.compile(ver)
        except ValueError as e:
            op.uops_sha[ver] = re.search(
                r'uops_sha\["' + ver + r'"\]="([0-9a-f]+)"', str(e)).group(1)
        op.compile(ver)
    return op


_t = sq(Src0)
MISH_A = _register_op("MISH_A_DIFF15436", Spec(
    body=Src1 * ((_t * C0 + C1) * _t + C2),
    reference=lambda in0, in1, s0, s1, imm2:
        (in1 * ((s0 * in0.astype(np.float64) ** 2 + s1) * in0.astype(np.float64) ** 2 + imm2)).astype(np.float32),
))
_t2 = sq(Src0)
MISH_B = _register_op("MISH_B_DIFF15436", Spec(
    body=Src1 * ((((_t2 + C0) * _t2 + C1) * _t2) + C2),
    reference=lambda in0, in1, s0, s1, imm2:
        (in1 * ((((in0.astype(np.float64) ** 2 + s0) * in0.astype(np.float64) ** 2 + s1) * in0.astype(np.float64) ** 2) + imm2)).astype(np.float32),
))
# pre = z4*C0 + x*C1   (C0 = -c2[k] AP column, C1 = c1[k] AP column)
PREOP2 = _register_op("PREOP2_DIFF15436", Spec(
    body=Src0 * C0 + Src1 * C1,
    reference=lambda in0, in1, s0, s1, imm2:
        (in0 * s0 + in1 * s1).astype(np.float32),
))
# x_new = clip(pre, C1, imm2)*C0 + s2   (C0 = p1[k] AP column; C1=-1, imm2=+1)
CLIPMA2 = _register_op("CLIPMA2_DIFF15436", Spec(
    body=minn(maxx(Src0, C1), C2) * C0 + Src1,
    reference=lambda in0, in1, s0, s1, imm2:
        (np.minimum(np.maximum(in0, s1), imm2) * s0 + in1).astype(np.float32),
))


# ---------------------------------------------------------------- schedule
def _vp_schedule():
    t = np.arange(1, T_STEPS + 1, dtype=np.float64)
    b_max, b_min = 10.0, 0.1
    alpha = np.exp(-b_min / T_STEPS - 0.5 * (b_max - b_min) * (2 * t - 1) / T_STEPS ** 2)
    betas = 1.0 - alpha
    ac = np.cumprod(1.0 - betas)
    ac_prev = np.concatenate([[1.0], ac[:-1]])
    return {
        'c1': np.sqrt(1.0 / ac).astype(np.float32),
        'c2': np.sqrt(1.0 / ac - 1.0).astype(np.float32),
        'p1': (betas * np.sqrt(ac_prev) / (1.0 - ac)).astype(np.float32),
        'p2': ((1.0 - ac_prev) * np.sqrt(1.0 - betas) / (1.0 - ac)).astype(np.float32),
        'logvar': np.log(np.clip(betas * (1.0 - ac_prev) / (1.0 - ac), 1e-20, None)).astype(np.float32),
    }


def _mish64(v):
    return v * np.tanh(np.logaddexp(0.0, v))


# ---------------------------------------------------------------- bass build
_CACHE = {}


def _build(nsteps=T_STEPS, use_b23=False, use_b4=False, repeats=1):
    key = ('nc', nsteps, use_b23, use_b4, repeats)
    if key in _CACHE:
        return _CACHE[key]

    nc = bacc.Bacc("TRN2", target_bir_lowering=False, debug=False, num_devices=NCORES)
    f32 = mybir.dt.float32
    bf = mybir.dt.bfloat16

    def din(name, shape, dt=f32):
        return nc.dram_tensor(name, shape, dt, kind="ExternalInput").ap()

    # bf16 blob: w1 (pad 128) | w2 | w3 | w4 | state (pad 128) | [mask4 | b23 pad | b4/ones2 pad]
    BFW = HIDDEN + 2 * HIDDEN + 2 * HIDDEN + 2 * ACTION_DIM + BPC
    if use_b23:
        BFW += 2 * BPC + 2 * 128
    if use_b4:
        BFW += ACTION_DIM + BPC
    d_bfb = din("bfblob", [128, BFW], bf)
    # f32 blob: temb [128, 200] | sched (pad 128 rows) [*, 400]
    d_f32b = din("f32blob", [128, 6 * T_STEPS + 1])
    d_xinit = din("x_init_t", [ACTION_DIM, BPC])
    d_noise = din("noise_t", [ACTION_DIM, T_STEPS * BPC])
    d_out = nc.dram_tensor("out_t", [ACTION_DIM, BPC], f32, kind="ExternalOutput").ap()

    with tile.TileContext(nc) as tc, ExitStack() as ctx:
        wp = ctx.enter_context(tc.tile_pool(name="weights", bufs=1))
        ap_ = ctx.enter_context(tc.tile_pool(name="acts", bufs=1))
        sp = ctx.enter_context(tc.tile_pool(name="small", bufs=1))
        pp = ctx.enter_context(tc.tile_pool(name="psum", bufs=1, space="PSUM"))

        bfb = wp.tile([128, BFW], bf, tag="bfb", name="bfb")
        nc.gpsimd.dma_start(bfb, d_bfb)
        off = 0
        w1 = bfb[0:KX, off:off + HIDDEN]; off += HIDDEN
        w2 = bfb[:, off:off + 2 * HIDDEN]; off += 2 * HIDDEN
        w3 = bfb[:, off:off + 2 * HIDDEN]; off += 2 * HIDDEN
        w4 = bfb[:, off:off + 2 * ACTION_DIM]; off += 2 * ACTION_DIM
        state_v = bfb[0:STATE_DIM, off:off + BPC]; off += BPC
        if use_b23:
            mask4 = bfb[0:4, off:off + 2 * BPC]; off += 2 * BPC
            b23 = bfb[0:4, off:off + 2 * 128]; off += 2 * 128
        if use_b4:
            b4hl = bfb[0:2, off:off + ACTION_DIM]; off += ACTION_DIM
            ones2 = bfb[0:2, off:off + BPC]; off += BPC

        f32b = wp.tile([128, 6 * T_STEPS + 1], f32, tag="f32b", name="f32b")
        nc.gpsimd.dma_start(f32b, d_f32b)
        temb_c = [f32b[:, 0:T_STEPS], f32b[:, T_STEPS:2 * T_STEPS]]
        s_p2 = f32b[0:ACTION_DIM, 2 * T_STEPS:3 * T_STEPS]
        s_c1 = f32b[0:ACTION_DIM, 3 * T_STEPS:4 * T_STEPS]
        s_nc2 = f32b[0:ACTION_DIM, 4 * T_STEPS:5 * T_STEPS]
        s_p1 = f32b[0:ACTION_DIM, 5 * T_STEPS:6 * T_STEPS]

        sig_bias = f32b[:, 6 * T_STEPS:6 * T_STEPS + 1]

        noise_sb = wp.tile([ACTION_DIM, T_STEPS * BPC], f32, tag="noise_sb", name="noise_sb")
        nc.gpsimd.dma_start(noise_sb, d_noise)
        noise3 = noise_sb.rearrange("p (k c) -> p k c", k=T_STEPS)

        hT = wp.tile([KX, BPC], bf, tag="hT", name="hT")
        nc.vector.tensor_copy(hT[0:STATE_DIM, :], state_v)
        xT = wp.tile([ACTION_DIM, BPC], f32, tag="xT", name="xT")
        nc.gpsimd.dma_start(xT, d_xinit)
        nc.vector.tensor_copy(hT[STATE_DIM:KX, :], xT)

        # persistent activation / psum tiles (static addresses inside the loop)
        h1 = ap_.tile([128, 2 * BPC], bf, tag="h1", name="h1")
        h2 = ap_.tile([128, 2 * BPC], bf, tag="h2", name="h2")
        h3 = ap_.tile([128, 2 * BPC], bf, tag="h3", name="h3")
        s_t = ap_.tile([128, 2 * BPC], f32, tag="s_t", name="s_t")
        wA = ap_.tile([128, 2 * BPC], f32, tag="wA", name="wA")
        s2 = sp.tile([ACTION_DIM, BPC], f32, tag="s2", name="s2")
        pre = sp.tile([ACTION_DIM, BPC], f32, tag="pre", name="pre")
        z1 = pp.tile([128, 2 * BPC], f32, tag="z1", name="z1")
        z2 = pp.tile([128, 2 * BPC], f32, tag="z2", name="z2")
        z3 = pp.tile([128, 2 * BPC], f32, tag="z3", name="z3")
        z4 = pp.tile([ACTION_DIM, BPC], f32, tag="z4", name="z4")

        SIG = mybir.ActivationFunctionType.Sigmoid
        MUL = mybir.AluOpType.mult
        ADD = mybir.AluOpType.add
        MAX = mybir.AluOpType.max
        MIN = mybir.AluOpType.min

        def mish(z, h):
            nc.scalar.activation(s_t, z, SIG, bias=sig_bias, scale=-A_S)
            nc.vector._custom_dve(MISH_A, out=wA, in0=s_t, in1=z, s0=QA, s1=QB, imm2=QC)
            nc.vector._custom_dve(MISH_B, out=h, in0=s_t, in1=wA, s0=MA, s1=MB, imm2=MC)

        import contextlib
        rep_cm = tc.For_i(0, repeats) if repeats > 1 else contextlib.nullcontext()
        with rep_cm, tc.For_i(0, nsteps) as k:
            kc1 = bass.ds(k, 1)

            # s2 = p2[k]*x + noise_k
            nc.vector.scalar_tensor_tensor(
                s2.rearrange("p (a c) -> p a c", a=1), xT.rearrange("p (a c) -> p a c", a=1),
                s_p2[:, kc1], noise3[:, kc1, :], MUL, ADD)

            # ---- L1: z1 = W1x^T [x; state]  + temb[k] ----
            for mc in (0, 1):
                nc.tensor.matmul(z1[:, mc * BPC:(mc + 1) * BPC],
                                 w1[:, mc * 128:(mc + 1) * 128], hT,
                                 start=True, stop=True)
                nc.vector.tensor_scalar_add(
                    z1[:, mc * BPC:(mc + 1) * BPC],
                    z1[:, mc * BPC:(mc + 1) * BPC],
                    temb_c[mc][:, kc1])
            mish(z1, h1)

            # ---- L2 / L3 ----
            for wd, hin, zt, hout, boff in ((w2, h1, z2, h2, 0), (w3, h2, z3, h3, 128)):
                if use_b23:
                    nc.tensor.matmul(zt, b23[0:4, boff:boff + 128], mask4,
                                     start=True, stop=False)
                for mc in (0, 1):
                    zslice = zt[:, mc * BPC:(mc + 1) * BPC]
                    for kc in (0, 1):
                        nc.tensor.matmul(
                            zslice,
                            wd[:, kc * HIDDEN + mc * 128:kc * HIDDEN + (mc + 1) * 128],
                            hin[:, kc * BPC:(kc + 1) * BPC],
                            start=(kc == 0 and not use_b23), stop=(kc == 1))
                mish(zt, hout)

            # ---- L4: eps psum [16, BPC] ----
            if use_b4:
                nc.tensor.matmul(z4, b4hl, ones2, start=True, stop=False)
            nc.tensor.matmul(z4, w4[:, 0:ACTION_DIM], h3[:, 0:BPC],
                             start=not use_b4, stop=False)
            nc.tensor.matmul(z4, w4[:, ACTION_DIM:2 * ACTION_DIM], h3[:, BPC:2 * BPC],
                             start=False, stop=True)

            # ---- x update ----
            nc.vector._custom_dve(PREOP2, out=pre, in0=z4, in1=xT,
                                  s0=s_nc2[:, kc1], s1=s_c1[:, kc1])
            nc.vector._custom_dve(CLIPMA2, out=xT, in0=pre, in1=s2,
                                  s0=s_p1[:, kc1],
                                  s1=-1.0, imm2=1.0)
            nc.vector.tensor_copy(hT[STATE_DIM:KX, :], xT)

        out_f = sp.tile([ACTION_DIM, BPC], f32, tag="out_f", name="out_f")
        nc.vector.tensor_scalar(out_f, xT, -1.0, 1.0, MAX, MIN)
        nc.sync.dma_start(d_out, out_f)

    nc.compile()
    _CACHE[key] = nc
    return nc


# ---------------------------------------------------------------- host side
def _host_prep(inputs):
    sched = _vp_schedule()
    f64 = np.float64

    W1 = np.asarray(inputs['W1'], np.float32)
    b1 = np.asarray(inputs['b1'], np.float32)
    W2 = np.asarray(inputs['W2'], np.float32)
    b2 = np.asarray(inputs['b2'], np.float32)
    W3 = np.asarray(inputs['W3'], np.float32)
    b3 = np.asarray(inputs['b3'], np.float32)
    W4 = np.asarray(inputs['W4'], np.float32)
    b4 = np.asarray(inputs['b4'], np.float32)

    # time-embedding MLP for all 100 timesteps (host, float64)
    half = TIME_DIM // 2
    freqs = np.exp(np.arange(half, dtype=f64) * (-math.log(10000.0) / (half - 1)))
    ivals = np.arange(T_STEPS, dtype=f64)
    ang = ivals[:, None] * freqs[None, :]
    emb = np.concatenate([np.sin(ang), np.cos(ang)], axis=1)
    t1 = _mish64(emb @ np.asarray(inputs['time_W1'], f64) + np.asarray(inputs['time_b1'], f64))
    temb = t1 @ np.asarray(inputs['time_W2'], f64) + np.asarray(inputs['time_b2'], f64)

    # beta-folded biases (the quintic mish fit is exact-mish + BETA; fold the
    # constant BETA into the next layer's bias)
    b2e = (b2.astype(f64) + BETA * W2.astype(f64).sum(axis=0)).astype(np.float32)
    b3e = (b3.astype(f64) + BETA * W3.astype(f64).sum(axis=0)).astype(np.float32)
    b4e = (b4.astype(f64) + BETA * W4.astype(f64).sum(axis=0)).astype(np.float32)

    # contrib[i] = temb[i] @ W1[16:48] + b1  -> per-step L1 bias, f32 exact
    contrib = (temb @ W1[16:48].astype(f64) + b1.astype(f64)).astype(np.float32)  # [100, 256] by timestep i

    ik = T_STEPS - 1 - np.arange(T_STEPS)   # timestep for loop iteration k
    # temb table by k: [128, 2*100] (chunk0 | chunk1)
    ck = contrib[ik]                        # [100, 256] by k
    temb_t = np.concatenate([ck[:, 0:128].T, ck[:, 128:256].T], axis=1).astype(np.float32)

    # schedule tables by k, replicated over the 16 feature partitions:
    # [16, 4*100] = p2 | c1 | -c2 | p1
    c1k = sched['c1'][ik]; c2k = sched['c2'][ik]
    p1k = sched['p1'][ik]; p2k = sched['p2'][ik]
    sched_t = np.concatenate([
        np.tile(p2k, (ACTION_DIM, 1)),
        np.tile(c1k, (ACTION_DIM, 1)),
        np.tile(-c2k, (ACTION_DIM, 1)),
        np.tile(p1k, (ACTION_DIM, 1)),
    ], axis=1).astype(np.float32)

    def hilo(v):
        v32 = np.asarray(v, np.float32)
        hi = v32.astype(BF16)
        lo = (v32 - hi.astype(np.float32)).astype(BF16)
        return hi, lo

    w1x = np.concatenate([W1[48:112], W1[0:16]], axis=0)  # rows = [state; x]
    w1_t = w1x.astype(BF16)
    # W2/W3 packed [(kc), 128, (mc)] -> [128, 2*256]: cols kc*256+mc*128
    def pack_w(W):
        out = np.zeros((128, 2 * HIDDEN), np.float32)
        for kc in (0, 1):
            for mc in (0, 1):
                out[:, kc * HIDDEN + mc * 128:kc * HIDDEN + (mc + 1) * 128] = \
                    W[kc * 128:(kc + 1) * 128, mc * 128:(mc + 1) * 128]
        return out.astype(BF16)
    w2_t = pack_w(W2)
    w3_t = pack_w(W3)
    w4_t = np.concatenate([W4[0:128], W4[128:256]], axis=1).astype(BF16)

    use_b23 = bool(max(np.abs(b2e).max(), np.abs(b3e).max()) > 1e-7)
    use_b4 = bool(np.abs(b4e).max() > 1e-7)

    # f32 blob: temb [128, 200] | sched [16->128 pad, 400] | sigmoid bias col
    f32blob = np.zeros((128, 6 * T_STEPS + 1), np.float32)
    f32blob[:, 0:2 * T_STEPS] = temb_t
    f32blob[0:ACTION_DIM, 2 * T_STEPS:6 * T_STEPS] = sched_t
    f32blob[:, 6 * T_STEPS] = -D_S

    # per-step noise scaling (fp32, matching the reference ops)
    sig = np.exp(0.5 * sched['logvar']).astype(np.float32)  # [100] by timestep i
    scale = sig[ik] * (ik != 0).astype(np.float32)          # [100] by k
    noise = np.asarray(inputs['noise'], np.float32)
    noise_scaled = noise * scale[:, None, None]             # [100, B, 16]

    state = np.asarray(inputs['state'], np.float32)
    x_init = np.asarray(inputs['x_init'], np.float32)

    BFW = HIDDEN + 2 * HIDDEN + 2 * HIDDEN + 2 * ACTION_DIM + BPC
    if use_b23:
        BFW += 2 * BPC + 2 * 128
    if use_b4:
        BFW += ACTION_DIM + BPC

    in_maps = []
    for c in range(NCORES):
        sl = slice(c * BPC, (c + 1) * BPC)
        blob = np.zeros((128, BFW), np.float32)
        off = 0
        blob[0:KX, off:off + HIDDEN] = w1x; off += HIDDEN
        blob[:, off:off + 2 * HIDDEN] = np.asarray(w2_t, np.float32); off += 2 * HIDDEN
        blob[:, off:off + 2 * HIDDEN] = np.asarray(w3_t, np.float32); off += 2 * HIDDEN
        blob[:, off:off + 2 * ACTION_DIM] = np.asarray(w4_t, np.float32); off += 2 * ACTION_DIM
        blob[0:STATE_DIM, off:off + BPC] = state[sl].T; off += BPC
        if use_b23:
            mask4 = np.zeros((4, 2 * BPC), np.float32)
            mask4[0:2, :BPC] = 1.0
            mask4[2:4, BPC:] = 1.0
            blob[0:4, off:off + 2 * BPC] = mask4; off += 2 * BPC
            hi, lo = hilo(np.stack([b2e, b3e]))
            hi = hi.astype(np.float32); lo = lo.astype(np.float32)
            b23p = np.stack([hi[:, :128], lo[:, :128], hi[:, 128:], lo[:, 128:]], axis=0).reshape(4, -1)
            blob[0:4, off:off + 2 * 128] = b23p; off += 2 * 128
        if use_b4:
            hi, lo = hilo(b4e)
            blob[0:2, off:off + ACTION_DIM] = np.stack([hi, lo]).astype(np.float32); off += ACTION_DIM
            blob[0:2, off:off + BPC] = 1.0; off += BPC
        m = dict(
            bfblob=blob.astype(BF16),
            f32blob=f32blob,
            x_init_t=np.ascontiguousarray(x_init[sl].T),
            # noise3[p, k, c] = noise_scaled[k, batch c, feature p]
            noise_t=np.ascontiguousarray(
                noise_scaled[:, sl, :].transpose(2, 0, 1).reshape(ACTION_DIM, T_STEPS * BPC)),
        )
        in_maps.append(m)
    return in_maps, use_b23, use_b4


def run(inputs, trace=False, nsteps=T_STEPS):
    in_maps, use_b23, use_b4 = _host_prep(inputs)
    nc = _build(nsteps, use_b23, use_b4)
    res = bass_utils.run_bass_kernel_spmd(
        nc, in_maps, core_ids=list(range(NCORES)), trace=trace)
    out = np.empty((BATCH, ACTION_DIM), np.float32)
    for c in range(NCORES):
        out[c * BPC:(c + 1) * BPC] = res.results[c]['out_t'].T
    return out, res


def kernel(**inputs) -> np.ndarray:
    out, _ = run(inputs, trace=False)
    return out


# revision 22
# speedup vs baseline: 1.2517x; 1.2517x over previous
"""Trainium2 Bass kernel for nn_Diffusion_15436112462451.

Strategy: pure data parallelism over the batch (2048 -> 8 cores x 256),
feature-major activations on-chip, and -- the key change vs the unrolled
baseline -- the 100-step denoising loop runs as a single For_i HARDWARE
loop.  The execution environment charges a large fixed cost per STATIC
program instruction (~60us each; measured: an unrolled 2900-instruction
program costs ~183ms while the same work inside a hardware loop is
dominated by true device time ~1ms).  The loop body is ~29 static
instructions; all step-varying quantities are indexed with register-based
dynamic APs (bass.ds) off per-step SBUF tables:

  - noise        [16, 100, BPC] f32, slice [:, ds(k,1), :]
  - temb contrib [128, 100] f32 x2 chunks, column ds(k,1) added to the L1
    PSUM with a per-partition tensor_scalar add (f32 -- exact bias)
  - schedule scalars c1/-c2/p1/p2: [16, 100] f32 tables, column ds(k,1),
    consumed as per-partition scalar APs by the x-update DVE ops

Per step: 12 bf16 matmuls on PE (L1 K=80 x2, L2/L3 4 each, L4 2; biases
b2/b3/b4 ride a mask rank-4 matmul / rank-2 prime only when nonzero --
they are zero in this problem), 2 temb-column adds + 3 sigmoid passes on
ScalarE/DVE, 6 custom-DVE mish-completion passes (exact-mish quintic in
t = sigmoid(-az-d)^2, max err 6.3e-5), and a 4-op x-update on DVE
(s2 / PREOP2 / CLIPMA2 / bf16-copy; the Pool engine rejects the
register-pointer TensorScalarPtr variants, and an in-place ScalarE
Identity-with-dynamic-bias on PSUM computes wrong results -- both
learned the hard way, keep these on the DVE).  The x iterate is kept in
f32 (bf16-only x fails: rel err 5.5e-2 vs 8.5e-4 with f32 master).  The
time-embedding MLP is batch-independent and precomputed on the host into
the [100,256] contrib table.  All constant inputs ride in two packed
blobs (bf16 weights+state / f32 tables) = 4 preamble DMAs total;
static program size is ~199 instructions (vs ~2950 unrolled), which
dominates the measured device span in this environment.
"""
import sys
import math
import re
import numpy as np

for _p in ('/opt/trn_rl_repo', '/root/.axon_site/_ro/trn_rl_repo'):
    if _p not in sys.path:
        sys.path.insert(0, _p)

import ml_dtypes
from contextlib import ExitStack
import concourse.bass as bass
from concourse import bacc
from concourse import mybir, tile, bass_utils, dve_ops
from concourse.dve_spec import Spec, Src0, Src1, C0, C1, C2, sq, maxx, minn

BF16 = ml_dtypes.bfloat16
NCORES = 8
BATCH = 2048
BPC = BATCH // NCORES          # 256 batch rows per core
T_STEPS = 100
STATE_DIM, ACTION_DIM, HIDDEN, TIME_DIM = 64, 16, 256, 32
KX = ACTION_DIM + STATE_DIM    # 80 rows of W1 used for [x; state]

# --- activation fit constants (deg-5 sigma-poly factorization) ---
A_S = 0.9990298806699722
D_S = -0.0005000143935776705
BETA = 4.708088756431602e-05
QA, QB, QC = -0.21302398380145082, 0.6455208072356895, -0.6201860532189531
MA, MB, MC = -0.9194163848641597, 1.5334239721923986, -1.6124382654378613


# ---------------------------------------------------------------- custom ops
def _register_op(name, spec):
    for op in dve_ops.OPS:
        if op.name == name:
            return op
    op = dve_ops.DveOp(name, spec, False, uops_sha={"v3": "?", "v4": "?"})
    dve_ops.OPS.append(op)
    dve_ops.CUSTOM_DVE_SPECS[name] = spec
    dve_ops._SUB_OPCODE_FOR_NAME[name] = (
        dve_ops._CUSTOM_DVE_ROW_BASE + len(dve_ops.OPS) - 1)
    for ver in ("v3", "v4"):
        try:
            op.compile(ver)
        except ValueError as e:
            op.uops_sha[ver] = re.search(
                r'uops_sha\["' + ver + r'"\]="([0-9a-f]+)"', str(e)).group(1)
        op.compile(ver)
    return op


_t = sq(Src0)
MISH_A = _register_op("MISH_A_DIFF15436", Spec(
    body=Src1 * ((_t * C0 + C1) * _t + C2),
    reference=lambda in0, in1, s0, s1, imm2:
        (in1 * ((s0 * in0.astype(np.float64) ** 2 + s1) * in0.astype(np.float64) ** 2 + imm2)).astype(np.float32),
))
_t2 = sq(Src0)
MISH_B = _register_op("MISH_B_DIFF15436", Spec(
    body=Src1 * ((((_t2 + C0) * _t2 + C1) * _t2) + C2),
    reference=lambda in0, in1, s0, s1, imm2:
        (in1 * ((((in0.astype(np.float64) ** 2 + s0) * in0.astype(np.float64) ** 2 + s1) * in0.astype(np.float64) ** 2) + imm2)).astype(np.float32),
))
# pre = z4*C0 + x*C1   (C0 = -c2[k] AP column, C1 = c1[k] AP column)
PREOP2 = _register_op("PREOP2_DIFF15436", Spec(
    body=Src0 * C0 + Src1 * C1,
    reference=lambda in0, in1, s0, s1, imm2:
        (in0 * s0 + in1 * s1).astype(np.float32),
))
# x_new = clip(pre, C1, imm2)*C0 + s2   (C0 = p1[k] AP column; C1=-1, imm2=+1)
CLIPMA2 = _register_op("CLIPMA2_DIFF15436", Spec(
    body=minn(maxx(Src0, C1), C2) * C0 + Src1,
    reference=lambda in0, in1, s0, s1, imm2:
        (np.minimum(np.maximum(in0, s1), imm2) * s0 + in1).astype(np.float32),
))


# ---------------------------------------------------------------- schedule
def _vp_schedule():
    t = np.arange(1, T_STEPS + 1, dtype=np.float64)
    b_max, b_min = 10.0, 0.1
    alpha = np.exp(-b_min / T_STEPS - 0.5 * (b_max - b_min) * (2 * t - 1) / T_STEPS ** 2)
    betas = 1.0 - alpha
    ac = np.cumprod(1.0 - betas)
    ac_prev = np.concatenate([[1.0], ac[:-1]])
    return {
        'c1': np.sqrt(1.0 / ac).astype(np.float32),
        'c2': np.sqrt(1.0 / ac - 1.0).astype(np.float32),
        'p1': (betas * np.sqrt(ac_prev) / (1.0 - ac)).astype(np.float32),
        'p2': ((1.0 - ac_prev) * np.sqrt(1.0 - betas) / (1.0 - ac)).astype(np.float32),
        'logvar': np.log(np.clip(betas * (1.0 - ac_prev) / (1.0 - ac), 1e-20, None)).astype(np.float32),
    }


def _mish64(v):
    return v * np.tanh(np.logaddexp(0.0, v))


# ---------------------------------------------------------------- bass build
_CACHE = {}


def _build(nsteps=T_STEPS, use_b23=False, use_b4=False, repeats=1):
    key = ('nc', nsteps, use_b23, use_b4, repeats)
    if key in _CACHE:
        return _CACHE[key]

    nc = bacc.Bacc("TRN2", target_bir_lowering=False, debug=False, num_devices=NCORES)
    f32 = mybir.dt.float32
    bf = mybir.dt.bfloat16

    def din(name, shape, dt=f32):
        return nc.dram_tensor(name, shape, dt, kind="ExternalInput").ap()

    # bf16 blob: w1 (pad 128) | w2 | w3 | w4 | state (pad 128) | [mask4 | b23 pad | b4/ones2 pad]
    BFW = HIDDEN + 2 * HIDDEN + 2 * HIDDEN + 2 * ACTION_DIM + BPC
    if use_b23:
        BFW += 2 * BPC + 2 * 128
    if use_b4:
        BFW += ACTION_DIM + BPC
    d_bfb = din("bfblob", [128, BFW], bf)
    # f32 blob: temb [128, 200] | sched (pad 128 rows) [*, 400]
    d_f32b = din("f32blob", [128, 6 * T_STEPS + 1 + BPC])
    d_noise = din("noise_t", [ACTION_DIM, T_STEPS * BPC])
    d_out = nc.dram_tensor("out_t", [ACTION_DIM, BPC], f32, kind="ExternalOutput").ap()

    with tile.TileContext(nc) as tc, ExitStack() as ctx:
        wp = ctx.enter_context(tc.tile_pool(name="weights", bufs=1))
        ap_ = ctx.enter_context(tc.tile_pool(name="acts", bufs=1))
        sp = ctx.enter_context(tc.tile_pool(name="small", bufs=1))
        pp = ctx.enter_context(tc.tile_pool(name="psum", bufs=1, space="PSUM"))

        bfb = wp.tile([128, BFW], bf, tag="bfb", name="bfb")
        nc.gpsimd.dma_start(bfb, d_bfb)
        off = 0
        w1 = bfb[0:KX, off:off + HIDDEN]; off += HIDDEN
        w2 = bfb[:, off:off + 2 * HIDDEN]; off += 2 * HIDDEN
        w3 = bfb[:, off:off + 2 * HIDDEN]; off += 2 * HIDDEN
        w4 = bfb[:, off:off + 2 * ACTION_DIM]; off += 2 * ACTION_DIM
        state_v = bfb[0:STATE_DIM, off:off + BPC]; off += BPC
        if use_b23:
            mask4 = bfb[0:4, off:off + 2 * BPC]; off += 2 * BPC
            b23 = bfb[0:4, off:off + 2 * 128]; off += 2 * 128
        if use_b4:
            b4hl = bfb[0:2, off:off + ACTION_DIM]; off += ACTION_DIM
            ones2 = bfb[0:2, off:off + BPC]; off += BPC

        f32b = wp.tile([128, 6 * T_STEPS + 1 + BPC], f32, tag="f32b", name="f32b")
        nc.gpsimd.dma_start(f32b, d_f32b)
        temb_c = [f32b[:, 0:T_STEPS], f32b[:, T_STEPS:2 * T_STEPS]]
        s_p2 = f32b[0:ACTION_DIM, 2 * T_STEPS:3 * T_STEPS]
        s_c1 = f32b[0:ACTION_DIM, 3 * T_STEPS:4 * T_STEPS]
        s_nc2 = f32b[0:ACTION_DIM, 4 * T_STEPS:5 * T_STEPS]
        s_p1 = f32b[0:ACTION_DIM, 5 * T_STEPS:6 * T_STEPS]

        sig_bias = f32b[:, 6 * T_STEPS:6 * T_STEPS + 1]

        noise_sb = wp.tile([ACTION_DIM, T_STEPS * BPC], f32, tag="noise_sb", name="noise_sb")
        nc.gpsimd.dma_start(noise_sb, d_noise)
        noise3 = noise_sb.rearrange("p (k c) -> p k c", k=T_STEPS)

        hT = wp.tile([KX, BPC], bf, tag="hT", name="hT")
        nc.vector.tensor_copy(hT[0:STATE_DIM, :], state_v)
        xT = f32b[0:ACTION_DIM, 6 * T_STEPS + 1:6 * T_STEPS + 1 + BPC]
        nc.vector.tensor_copy(hT[STATE_DIM:KX, :], xT)

        # persistent activation / psum tiles (static addresses inside the loop)
        h1 = ap_.tile([128, 2 * BPC], bf, tag="h1", name="h1")
        h2 = ap_.tile([128, 2 * BPC], bf, tag="h2", name="h2")
        h3 = ap_.tile([128, 2 * BPC], bf, tag="h3", name="h3")
        s_t = ap_.tile([128, 2 * BPC], f32, tag="s_t", name="s_t")
        wA = ap_.tile([128, 2 * BPC], f32, tag="wA", name="wA")
        s2 = sp.tile([ACTION_DIM, BPC], f32, tag="s2", name="s2")
        pre = sp.tile([ACTION_DIM, BPC], f32, tag="pre", name="pre")
        z1 = pp.tile([128, 2 * BPC], f32, tag="z1", name="z1")
        z2 = pp.tile([128, 2 * BPC], f32, tag="z2", name="z2")
        z3 = pp.tile([128, 2 * BPC], f32, tag="z3", name="z3")
        z4 = pp.tile([ACTION_DIM, BPC], f32, tag="z4", name="z4")

        SIG = mybir.ActivationFunctionType.Sigmoid
        MUL = mybir.AluOpType.mult
        ADD = mybir.AluOpType.add
        MAX = mybir.AluOpType.max
        MIN = mybir.AluOpType.min

        def mish(z, h):
            nc.scalar.activation(s_t, z, SIG, bias=sig_bias, scale=-A_S)
            nc.vector._custom_dve(MISH_A, out=wA, in0=s_t, in1=z, s0=QA, s1=QB, imm2=QC)
            nc.vector._custom_dve(MISH_B, out=h, in0=s_t, in1=wA, s0=MA, s1=MB, imm2=MC)

        import contextlib
        rep_cm = tc.For_i(0, repeats) if repeats > 1 else contextlib.nullcontext()
        with rep_cm, tc.For_i(0, nsteps) as k:
            kc1 = bass.ds(k, 1)

            # s2 = p2[k]*x + noise_k
            nc.vector.scalar_tensor_tensor(
                s2.rearrange("p (a c) -> p a c", a=1), xT.rearrange("p (a c) -> p a c", a=1),
                s_p2[:, kc1], noise3[:, kc1, :], MUL, ADD)

            # ---- L1: z1 = W1x^T [x; state]  + temb[k] ----
            for mc in (0, 1):
                nc.tensor.matmul(z1[:, mc * BPC:(mc + 1) * BPC],
                                 w1[:, mc * 128:(mc + 1) * 128], hT,
                                 start=True, stop=True)
                nc.vector.tensor_scalar_add(
                    z1[:, mc * BPC:(mc + 1) * BPC],
                    z1[:, mc * BPC:(mc + 1) * BPC],
                    temb_c[mc][:, kc1])
            mish(z1, h1)

            # ---- L2 / L3 ----
            for wd, hin, zt, hout, boff in ((w2, h1, z2, h2, 0), (w3, h2, z3, h3, 128)):
                if use_b23:
                    nc.tensor.matmul(zt, b23[0:4, boff:boff + 128], mask4,
                                     start=True, stop=False)
                for mc in (0, 1):
                    zslice = zt[:, mc * BPC:(mc + 1) * BPC]
                    for kc in (0, 1):
                        nc.tensor.matmul(
                            zslice,
                            wd[:, kc * HIDDEN + mc * 128:kc * HIDDEN + (mc + 1) * 128],
                            hin[:, kc * BPC:(kc + 1) * BPC],
                            start=(kc == 0 and not use_b23), stop=(kc == 1))
                mish(zt, hout)

            # ---- L4: eps psum [16, BPC] ----
            if use_b4:
                nc.tensor.matmul(z4, b4hl, ones2, start=True, stop=False)
            nc.tensor.matmul(z4, w4[:, 0:ACTION_DIM], h3[:, 0:BPC],
                             start=not use_b4, stop=False)
            nc.tensor.matmul(z4, w4[:, ACTION_DIM:2 * ACTION_DIM], h3[:, BPC:2 * BPC],
                             start=False, stop=True)

            # ---- x update ----
            nc.vector._custom_dve(PREOP2, out=pre, in0=z4, in1=xT,
                                  s0=s_nc2[:, kc1], s1=s_c1[:, kc1])
            nc.vector._custom_dve(CLIPMA2, out=xT, in0=pre, in1=s2,
                                  s0=s_p1[:, kc1],
                                  s1=-1.0, imm2=1.0)
            nc.vector.tensor_copy(hT[STATE_DIM:KX, :], xT)

        out_f = sp.tile([ACTION_DIM, BPC], f32, tag="out_f", name="out_f")
        nc.vector.tensor_scalar(out_f, xT, -1.0, 1.0, MAX, MIN)
        nc.sync.dma_start(d_out, out_f)

    nc.compile()
    _CACHE[key] = nc
    return nc


# ---------------------------------------------------------------- host side
def _host_prep(inputs):
    sched = _vp_schedule()
    f64 = np.float64

    W1 = np.asarray(inputs['W1'], np.float32)
    b1 = np.asarray(inputs['b1'], np.float32)
    W2 = np.asarray(inputs['W2'], np.float32)
    b2 = np.asarray(inputs['b2'], np.float32)
    W3 = np.asarray(inputs['W3'], np.float32)
    b3 = np.asarray(inputs['b3'], np.float32)
    W4 = np.asarray(inputs['W4'], np.float32)
    b4 = np.asarray(inputs['b4'], np.float32)

    # time-embedding MLP for all 100 timesteps (host, float64)
    half = TIME_DIM // 2
    freqs = np.exp(np.arange(half, dtype=f64) * (-math.log(10000.0) / (half - 1)))
    ivals = np.arange(T_STEPS, dtype=f64)
    ang = ivals[:, None] * freqs[None, :]
    emb = np.concatenate([np.sin(ang), np.cos(ang)], axis=1)
    t1 = _mish64(emb @ np.asarray(inputs['time_W1'], f64) + np.asarray(inputs['time_b1'], f64))
    temb = t1 @ np.asarray(inputs['time_W2'], f64) + np.asarray(inputs['time_b2'], f64)

    # beta-folded biases (the quintic mish fit is exact-mish + BETA; fold the
    # constant BETA into the next layer's bias)
    b2e = (b2.astype(f64) + BETA * W2.astype(f64).sum(axis=0)).astype(np.float32)
    b3e = (b3.astype(f64) + BETA * W3.astype(f64).sum(axis=0)).astype(np.float32)
    b4e = (b4.astype(f64) + BETA * W4.astype(f64).sum(axis=0)).astype(np.float32)

    # contrib[i] = temb[i] @ W1[16:48] + b1  -> per-step L1 bias, f32 exact
    contrib = (temb @ W1[16:48].astype(f64) + b1.astype(f64)).astype(np.float32)  # [100, 256] by timestep i

    ik = T_STEPS - 1 - np.arange(T_STEPS)   # timestep for loop iteration k
    # temb table by k: [128, 2*100] (chunk0 | chunk1)
    ck = contrib[ik]                        # [100, 256] by k
    temb_t = np.concatenate([ck[:, 0:128].T, ck[:, 128:256].T], axis=1).astype(np.float32)

    # schedule tables by k, replicated over the 16 feature partitions:
    # [16, 4*100] = p2 | c1 | -c2 | p1
    c1k = sched['c1'][ik]; c2k = sched['c2'][ik]
    p1k = sched['p1'][ik]; p2k = sched['p2'][ik]
    sched_t = np.concatenate([
        np.tile(p2k, (ACTION_DIM, 1)),
        np.tile(c1k, (ACTION_DIM, 1)),
        np.tile(-c2k, (ACTION_DIM, 1)),
        np.tile(p1k, (ACTION_DIM, 1)),
    ], axis=1).astype(np.float32)

    def hilo(v):
        v32 = np.asarray(v, np.float32)
        hi = v32.astype(BF16)
        lo = (v32 - hi.astype(np.float32)).astype(BF16)
        return hi, lo

    w1x = np.concatenate([W1[48:112], W1[0:16]], axis=0)  # rows = [state; x]
    w1_t = w1x.astype(BF16)
    # W2/W3 packed [(kc), 128, (mc)] -> [128, 2*256]: cols kc*256+mc*128
    def pack_w(W):
        out = np.zeros((128, 2 * HIDDEN), np.float32)
        for kc in (0, 1):
            for mc in (0, 1):
                out[:, kc * HIDDEN + mc * 128:kc * HIDDEN + (mc + 1) * 128] = \
                    W[kc * 128:(kc + 1) * 128, mc * 128:(mc + 1) * 128]
        return out.astype(BF16)
    w2_t = pack_w(W2)
    w3_t = pack_w(W3)
    w4_t = np.concatenate([W4[0:128], W4[128:256]], axis=1).astype(BF16)

    use_b23 = bool(max(np.abs(b2e).max(), np.abs(b3e).max()) > 1e-7)
    use_b4 = bool(np.abs(b4e).max() > 1e-7)

    # f32 blob: temb [128, 200] | sched [16->128 pad, 400] | sigmoid bias col
    f32blob = np.zeros((128, 6 * T_STEPS + 1 + BPC), np.float32)
    f32blob[:, 0:2 * T_STEPS] = temb_t
    f32blob[0:ACTION_DIM, 2 * T_STEPS:6 * T_STEPS] = sched_t
    f32blob[:, 6 * T_STEPS] = -D_S

    # per-step noise scaling (fp32, matching the reference ops)
    sig = np.exp(0.5 * sched['logvar']).astype(np.float32)  # [100] by timestep i
    scale = sig[ik] * (ik != 0).astype(np.float32)          # [100] by k
    noise = np.asarray(inputs['noise'], np.float32)
    noise_scaled = noise * scale[:, None, None]             # [100, B, 16]

    state = np.asarray(inputs['state'], np.float32)
    x_init = np.asarray(inputs['x_init'], np.float32)

    BFW = HIDDEN + 2 * HIDDEN + 2 * HIDDEN + 2 * ACTION_DIM + BPC
    if use_b23:
        BFW += 2 * BPC + 2 * 128
    if use_b4:
        BFW += ACTION_DIM + BPC

    in_maps = []
    for c in range(NCORES):
        sl = slice(c * BPC, (c + 1) * BPC)
        blob = np.zeros((128, BFW), np.float32)
        off = 0
        blob[0:KX, off:off + HIDDEN] = w1x; off += HIDDEN
        blob[:, off:off + 2 * HIDDEN] = np.asarray(w2_t, np.float32); off += 2 * HIDDEN
        blob[:, off:off + 2 * HIDDEN] = np.asarray(w3_t, np.float32); off += 2 * HIDDEN
        blob[:, off:off + 2 * ACTION_DIM] = np.asarray(w4_t, np.float32); off += 2 * ACTION_DIM
        blob[0:STATE_DIM, off:off + BPC] = state[sl].T; off += BPC
        if use_b23:
            mask4 = np.zeros((4, 2 * BPC), np.float32)
            mask4[0:2, :BPC] = 1.0
            mask4[2:4, BPC:] = 1.0
            blob[0:4, off:off + 2 * BPC] = mask4; off += 2 * BPC
            hi, lo = hilo(np.stack([b2e, b3e]))
            hi = hi.astype(np.float32); lo = lo.astype(np.float32)
            b23p = np.stack([hi[:, :128], lo[:, :128], hi[:, 128:], lo[:, 128:]], axis=0).reshape(4, -1)
            blob[0:4, off:off + 2 * 128] = b23p; off += 2 * 128
        if use_b4:
            hi, lo = hilo(b4e)
            blob[0:2, off:off + ACTION_DIM] = np.stack([hi, lo]).astype(np.float32); off += ACTION_DIM
            blob[0:2, off:off + BPC] = 1.0; off += BPC
        f32c = f32blob.copy()
        f32c[0:ACTION_DIM, 6 * T_STEPS + 1:] = x_init[sl].T
        m = dict(
            bfblob=blob.astype(BF16),
            f32blob=f32c,
            # noise3[p, k, c] = noise_scaled[k, batch c, feature p]
            noise_t=np.ascontiguousarray(
                noise_scaled[:, sl, :].transpose(2, 0, 1).reshape(ACTION_DIM, T_STEPS * BPC)),
        )
        in_maps.append(m)
    return in_maps, use_b23, use_b4


def run(inputs, trace=False, nsteps=T_STEPS):
    in_maps, use_b23, use_b4 = _host_prep(inputs)
    nc = _build(nsteps, use_b23, use_b4)
    res = bass_utils.run_bass_kernel_spmd(
        nc, in_maps, core_ids=list(range(NCORES)), trace=trace)
    out = np.empty((BATCH, ACTION_DIM), np.float32)
    for c in range(NCORES):
        out[c * BPC:(c + 1) * BPC] = res.results[c]['out_t'].T
    return out, res


def kernel(**inputs) -> np.ndarray:
    out, _ = run(inputs, trace=False)
    return out
